# revision 31
# baseline (speedup 1.0000x reference)
"""Causal multi-head attention (B=4, S=2048, D=1024, H=16, Dh=64) on 8 TRN2
NeuronCores.

Sharding: core c -> batch b = c//2, head-group g = c%2 (8 heads each).
W_q/W_k/W_v column-parallel (512 cols per core), W_o row-parallel (512 rows).
Each core computes a partial O^T [1024, 2048] for its batch; host sums the
two head-group partials per batch and transposes back to [S, D].

v2 pipeline (all matmul inputs bf16, cast on host):
  x^T via hardware DMA-XBAR transpose straight from DRAM (no PE transposes)
  Q^T/K^T = W^T x^T, V = x W_v -> persistent bf16 tiles
  attention per s-block, head pairs at PE row groups 0:64 / 64:128:
    paired score matmuls (K=64, concurrent row groups) -> one wide PSUM tile
    one merged exp over both heads' scores (ACT), causal-trimmed widths
    affine_select triangle mask only on the 128x128 diagonal squares
    AV accumulation with [V|1] ones column -> [65, 512] PSUM per head
    normalize: reciprocal_approx_fast + ones-matmul broadcast + DVE mul
  W_o projection + output DMA
  The W/V projections of s-block sb+1 and output projection of sb-1 are
  interleaved one matmul at a time into the attention loop of s-block sb to
  fill PE bubbles (keeps the HAM clock gate warm).
"""

from contextlib import ExitStack

import numpy as np
import ml_dtypes

import concourse.bass as bass
import concourse.mybir as mybir
import concourse.tile as tile
from concourse import bacc
from concourse.bass_utils import run_bass_kernel_spmd

F32 = mybir.dt.float32
BF16 = mybir.dt.bfloat16

S = 2048          # sequence length
D = 1024          # model dim
HL = 8            # local heads per core
DH = 64           # head dim
CL = HL * DH      # local cols (512)
SBS = 512         # s-block size
NSB = S // SBS    # 4 s-blocks
TS = 128          # tile size (partitions)
TTR = SBS // TS   # t-tiles per s-block (4)
N_CORES = 8

EXP = mybir.ActivationFunctionType.Exp
SCALE = 1.0 / 8.0  # 1/sqrt(DH)


def build_program():
    nc = bacc.Bacc(
        "TRN2", target_bir_lowering=False, debug=False, num_devices=N_CORES
    )
    x_d = nc.dram_tensor("x", [S, D], BF16, kind="ExternalInput").ap()
    wq_d = nc.dram_tensor("wq", [D, CL], BF16, kind="ExternalInput").ap()
    wk_d = nc.dram_tensor("wk", [D, CL], BF16, kind="ExternalInput").ap()
    wv_d = nc.dram_tensor("wv", [D, CL], BF16, kind="ExternalInput").ap()
    wo_d = nc.dram_tensor("wo", [CL, D], BF16, kind="ExternalInput").ap()
    out_d = nc.dram_tensor("out", [D, S], F32, kind="ExternalOutput").ap()

    with (
        tile.TileContext(nc) as tc,
        ExitStack() as ctx,
        nc.allow_low_precision(reason="bf16 attention, fp32 accumulation"),
    ):
        const = ctx.enter_context(tc.tile_pool(name="const", bufs=1))
        wpool = ctx.enter_context(tc.tile_pool(name="w", bufs=1))
        xtp = ctx.enter_context(tc.tile_pool(name="xt", bufs=1))
        ktp = ctx.enter_context(tc.tile_pool(name="kt", bufs=1))
        vp = ctx.enter_context(tc.tile_pool(name="v", bufs=1))
        qtp = ctx.enter_context(tc.tile_pool(name="qt", bufs=2))
        ptp = ctx.enter_context(tc.tile_pool(name="pt", bufs=3))
        ocp = ctx.enter_context(tc.tile_pool(name="oc", bufs=4))
        obp = ctx.enter_context(tc.tile_pool(name="ob", bufs=2))
        stp = ctx.enter_context(tc.tile_pool(name="st", bufs=4))
        mp = ctx.enter_context(tc.tile_pool(name="misc", bufs=8))
        psS = ctx.enter_context(tc.tile_pool(name="psS", bufs=2, space="PSUM"))
        psO = ctx.enter_context(tc.tile_pool(name="psO", bufs=2, space="PSUM"))
        psA = ctx.enter_context(tc.tile_pool(name="psA", bufs=2, space="PSUM"))

        # x^T: [128, 8 dtiles, 2048 s] via DMA-XBAR transpose, per (sb, dtile)
        # chunk so s-block 0's projections can start early.
        xt = xtp.tile([TS, 8, S], BF16)
        wq_s = wpool.tile([TS, 8, CL], BF16)
        wk_s = wpool.tile([TS, 8, CL], BF16)
        wv_s = wpool.tile([TS, 8, CL], BF16)
        # ALL dynamic DMAs serialize through one chain (cross-queue handoffs
        # cost ~1.3us each), so keep a single queue and order the chunks by
        # when the projection pipeline needs them: wq, x(sb0), wk, wv, then
        # the remaining x blocks, wo last.
        wo_s = wpool.tile([TS, 4, D], BF16)

        def x_chunks(sb):
            for j in range(8):
                nc.sync.dma_start(
                    out=xt[:, j, sb * SBS : (sb + 1) * SBS],
                    in_=x_d[sb * SBS : (sb + 1) * SBS, j * TS : (j + 1) * TS],
                    transpose=True,
                )

        for j in range(8):
            nc.sync.dma_start(wq_s[:, j, :], wq_d[j * TS : (j + 1) * TS, :])
        x_chunks(0)
        for w_s, w_d in ((wk_s, wk_d), (wv_s, wv_d)):
            for j in range(8):
                nc.sync.dma_start(
                    w_s[:, j, :], w_d[j * TS : (j + 1) * TS, :]
                )
        for sb in range(1, NSB):
            x_chunks(sb)
        for j in range(4):
            nc.sync.dma_start(wo_s[:, j, :], wo_d[j * TS : (j + 1) * TS, :])

        # K^T persistent: [128, 4 ctiles, 512]; head h -> ctile h//2, partition
        # offset (h%2)*64.  V persistent per head PAIR as [128, 4 stl, 4 pr,
        # 192]: cols 0:64 = v_even, col 64 = 1 (shared denominator column),
        # cols 65:128 = 1 (dead), cols 128:192 = v_odd.  The even head's AV
        # lhsT is cols 0:65 (M=65 -> AV rows 0..63, den row 64); the odd
        # head's is cols 64:192 (M=128 -> den row 0, AV rows 64..127), so the
        # pair's outputs concatenate to a full 128-partition tile with no
        # cross-partition moves.
        kt_r = [ktp.tile([TS, 4, SBS], BF16, name=f"kt_{r}") for r in range(NSB)]
        v_r = [
            vp.tile([TS, TTR, 4, 192], BF16, name=f"v_{r}") for r in range(NSB)
        ]
        for r in range(NSB):
            nc.vector.memset(v_r[r][:], 1.0)
        # all-ones [65+64, 64]: rows 0 and 64 serve as the [1,64] ones lhsT
        # for the reciprocal-broadcast matmuls at row groups 0 and 2.
        ones65 = const.tile([DH + 1, DH], F32)
        nc.vector.memset(ones65[:], 1.0)

        def proj_units(sb, qt):
            """B-stage for s-block sb: Q^T/K^T/V projections, one matmul per
            yield."""
            for ct in range(4):
                for w_s, isq in ((wq_s, True), (wk_s, False)):
                    ps = psA.tile(
                        [TS, SBS], F32, tag="ps", name=f"b_{sb}_{ct}_{int(isq)}"
                    )
                    for j in range(8):
                        nc.tensor.matmul(
                            ps[:],
                            w_s[:, j, ct * TS : (ct + 1) * TS],
                            xt[:, j, sb * SBS : (sb + 1) * SBS],
                            start=(j == 0),
                            stop=(j == 7),
                        )
                        yield
                    dst = qt if isq else kt_r[sb]
                    nc.vector.tensor_copy(dst[:, ct, :], ps[:])
            for stl in range(TTR):
                ps = psA.tile([TS, SBS], F32, tag="ps", name=f"bv_{sb}_{stl}")
                for j in range(8):
                    nc.tensor.matmul(
                        ps[:],
                        xt[:, j, sb * SBS + stl * TS : sb * SBS + (stl + 1) * TS],
                        wv_s[:, j, :],
                        start=(j == 0),
                        stop=(j == 7),
                    )
                    yield
                psv = ps[:].rearrange("p (pr two e) -> p pr two e", two=2, e=DH)
                nc.vector.tensor_copy(
                    v_r[sb][:, stl, :, 0:DH], psv[:, :, 0, :]
                )
                nc.vector.tensor_copy(
                    v_r[sb][:, stl, :, 2 * DH : 3 * DH], psv[:, :, 1, :]
                )

        def outproj_units(sb, oc, act_evac=False):
            """D-stage for s-block sb from its outcat^T tile, one matmul per
            yield.  act_evac=True moves the PSUM evacuation to the (by then
            idle) scalar engine and alternates output DMAs across both HWDGE
            queues — used for the final, serial output projection."""
            for mt in range(8):
                ps = psA.tile([TS, SBS], F32, tag="ps", name=f"d_{sb}_{mt}")
                for j in range(4):
                    nc.tensor.matmul(
                        ps[:],
                        wo_s[:, j, mt * TS : (mt + 1) * TS],
                        oc[:, j, :],
                        start=(j == 0),
                        stop=(j == 3),
                    )
                    yield
                ob = obp.tile([TS, SBS], F32)
                if act_evac:
                    nc.scalar.copy(ob[:], ps[:])
                else:
                    nc.vector.tensor_copy(ob[:], ps[:])
                nc.sync.dma_start(
                    out_d[mt * TS : (mt + 1) * TS, sb * SBS : (sb + 1) * SBS],
                    ob[:],
                )

        def chain(*gens):
            for g in gens:
                yield from g

        def take(gen, n):
            got = 0
            for _ in range(n):
                if next(gen, None) is None:
                    return got
                got += 1
            return got

        qt_tiles = [None] * NSB
        oc_tiles = [None] * NSB
        qt_tiles[0] = qtp.tile([TS, 4, SBS], BF16, tag="qt", name="qt_0")
        # s-block 0 projections run standalone (nothing to overlap yet)
        for _ in proj_units(0, qt_tiles[0]):
            pass

        for sb in range(NSB):
            qt = qt_tiles[sb]
            oc = ocp.tile([TS, 4, SBS], BF16, tag="oc", name=f"oc_{sb}")
            oc_tiles[sb] = oc
            # filler: projections of sb+1; all early output projections are
            # saved for the last s-block, whose attention is exp(ACT)-bound
            # and needs the extra matmuls to keep the PE duty (and the HAM
            # clock gate) up.
            gens = []
            n_units = 0
            if sb + 1 < NSB:
                qt_tiles[sb + 1] = qtp.tile(
                    [TS, 4, SBS], BF16, tag="qt", name=f"qt_{sb + 1}"
                )
                gens.append(proj_units(sb + 1, qt_tiles[sb + 1]))
                n_units += 96
            if sb == NSB - 1:
                for b in range(NSB - 1):
                    gens.append(outproj_units(b, oc_tiles[b]))
                    n_units += 32
            filler = chain(*gens)
            n_tis = 4 * (sb + 1) * TTR  # total AV steps this s-block

            ntt = (sb + 1) * TTR
            pending = []
            for hp in range(4):
                ct = hp
                tts = list(range(sb * TTR, ntt)) + list(range(0, sb * TTR))
                pos = [
                    psO.tile([TS, SBS], F32, tag="po", name=f"po_{sb}_{hp}_{i}")
                    for i in range(2)
                ]

                def scores(ti):
                    tt = tts[ti]
                    k = tt - sb * TTR if ti < TTR else None
                    s0 = TS * k if k is not None else 0
                    w = psS.tile(
                        [TS, 2, SBS], F32, tag="sc", name=f"sc_{sb}_{hp}_{ti}"
                    )
                    for i in range(2):
                        poff = DH * i
                        nc.tensor.matmul(
                            w[:, i, s0:SBS],
                            kt_r[tt // TTR][
                                poff : poff + DH,
                                ct,
                                (tt % TTR) * TS : (tt % TTR + 1) * TS,
                            ],
                            qt[poff : poff + DH, ct, s0:SBS],
                            start=True,
                            stop=True,
                        )
                    pt = ptp.tile([TS, 2, SBS], BF16)
                    nc.scalar.activation(
                        pt[:, :, s0:SBS], w[:, :, s0:SBS], EXP, scale=SCALE
                    )
                    if k is not None:
                        for i in range(2):
                            # triangle mask on the diagonal 128x128 square:
                            # keep where col >= partition
                            nc.gpsimd.affine_select(
                                out=pt[:, i, s0 : s0 + TS],
                                in_=pt[:, i, s0 : s0 + TS],
                                compare_op=mybir.AluOpType.is_ge,
                                fill=0.0,
                                base=0,
                                channel_multiplier=-1,
                                pattern=[[1, TS]],
                            )
                    return pt, s0

                prev = scores(0)
                for ti in range(len(tts)):
                    nxt = scores(ti + 1) if ti + 1 < len(tts) else None
                    pt, s0 = prev
                    tt = tts[ti]
                    vpr = v_r[tt // TTR][:, tt % TTR, hp]
                    nc.tensor.matmul(
                        pos[0][0 : DH + 1, s0:SBS],
                        vpr[:, 0 : DH + 1],
                        pt[:, 0, s0:SBS],
                        start=(ti == 0), stop=(ti == len(tts) - 1),
                    )
                    nc.tensor.matmul(
                        pos[1][:, s0:SBS],
                        vpr[:, DH:],
                        pt[:, 1, s0:SBS],
                        start=(ti == 0), stop=(ti == len(tts) - 1),
                    )
                    if pending:
                        pending.pop(0)()
                    # spread remaining filler units evenly over remaining
                    # AV steps (ceil so the generator drains by block end)
                    k = -(-n_units // n_tis) if n_tis > 0 else n_units
                    n_units -= take(filler, k)
                    n_tis -= 1
                    prev = nxt

                # Reciprocal of the denominator row straight from PSUM first
                # (fast approx, ~0.7us) so the deferred bc matmul on PE is
                # never blocked; then evacuate the [64, 512] accumulators so
                # the pos PSUM banks free.  Normalize tail (broadcast/multiply)
                # is deferred into the next head pair's loop.
                # Denominator rows (even head: pos0 row 64; odd head: pos1
                # row 0) -> one [2,512] reciprocal, issued first so the
                # deferred bc matmul on PE is never blocked; then evacuate
                # the pair's AV halves into one [128,512] tile.  The last
                # head pair evacuates on the by-then-idle scalar engine.
                last = sb == NSB - 1 and hp == 3
                # den rows live at partitions {0, 64} (engine writes must
                # start at a 0/32/64 partition base); the recip runs over all
                # 128 partitions (same cost, rows 1..63 are dead).
                den = mp.tile([TS, SBS], F32, tag="den", name=f"den_{sb}_{hp}")
                nc.vector.tensor_copy(den[0:1, :], pos[0][DH : DH + 1, :])
                nc.vector.tensor_copy(den[DH : DH + 1, :], pos[1][0:1, :])
                rs = mp.tile([TS, SBS], F32, tag="rs", name=f"rs_{sb}_{hp}")
                nc.vector.reciprocal_approx_fast(out=rs[:], in_=den[:])
                pn = stp.tile([TS, SBS], F32, tag="pn")
                if last:
                    nc.scalar.copy(pn[0:DH, :], pos[0][0:DH, :])
                    nc.scalar.copy(pn[DH:TS, :], pos[1][DH:TS, :])
                else:
                    nc.vector.tensor_copy(pn[0:DH, :], pos[0][0:DH, :])
                    nc.vector.tensor_copy(pn[DH:TS, :], pos[1][DH:TS, :])

                def norm_tail(pn, rs, ct=ct, sb=sb, hp=hp, oc=oc):
                    bc = psA.tile(
                        [TS, SBS], F32, tag="ps", name=f"bc_{sb}_{hp}"
                    )
                    # two concurrent-capable broadcasts: (row grp 0 -> cols
                    # 0:64) and (row grp 2 -> cols 64:128)
                    nc.tensor.matmul(
                        bc[0:DH, :], ones65[0:1, :], rs[0:1, :],
                        start=True, stop=True,
                    )
                    nc.tensor.matmul(
                        bc[DH:TS, :], ones65[DH : DH + 1, :],
                        rs[DH : DH + 1, :],
                        start=True, stop=True,
                    )
                    nc.vector.tensor_mul(oc[:, ct, :], pn[:], bc[:])

                pending.append(lambda pn=pn, rs=rs: norm_tail(pn, rs))

            # flush deferred normalize tails for the last head pair
            for u in pending:
                u()
            pending = []
            # drain remaining fillers before the next s-block needs qt/kt/v
            for _ in filler:
                pass

        # final output projection (nothing left to overlap with)
        for _ in outproj_units(NSB - 1, oc_tiles[NSB - 1], act_evac=True):
            pass

    nc.compile()
    return nc


_prog_cache = {}


def _get_program():
    if "p" not in _prog_cache:
        _prog_cache["p"] = build_program()
    return _prog_cache["p"]


def make_in_maps(inputs):
    bf = ml_dtypes.bfloat16
    x = np.asarray(inputs["x"], np.float32)
    wq = np.asarray(inputs["W_q"], np.float32)
    wk = np.asarray(inputs["W_k"], np.float32)
    wv = np.asarray(inputs["W_v"], np.float32)
    wo = np.asarray(inputs["W_o"], np.float32)
    in_maps = []
    for c in range(N_CORES):
        b, g = c // 2, c % 2
        cs = slice(g * CL, (g + 1) * CL)
        in_maps.append(
            {
                "x": np.ascontiguousarray(x[b]).astype(bf),
                "wq": np.ascontiguousarray(wq[:, cs]).astype(bf),
                "wk": np.ascontiguousarray(wk[:, cs]).astype(bf),
                "wv": np.ascontiguousarray(wv[:, cs]).astype(bf),
                "wo": np.ascontiguousarray(wo[cs, :]).astype(bf),
            }
        )
    return in_maps


def run(inputs, trace=False, **kwargs):
    nc = _get_program()
    res = run_bass_kernel_spmd(
        nc, make_in_maps(inputs), core_ids=list(range(N_CORES)),
        trace=trace, **kwargs
    )
    outs = [res.results[c]["out"] for c in range(N_CORES)]
    full = np.stack(
        [(outs[2 * b] + outs[2 * b + 1]).T for b in range(4)]
    ).astype(np.float32)
    return full, res


def kernel(**inputs) -> np.ndarray:
    out, _ = run(inputs)
    return out



# revision 34
# speedup vs baseline: 1.1780x; 1.1780x over previous
"""Causal multi-head attention (B=4, S=2048, D=1024, H=16, Dh=64) on 8 TRN2
NeuronCores.

Sharding: core c -> batch b = c//2, head-group g = c%2 (8 heads each).
W_q/W_k/W_v column-parallel (512 cols per core), W_o row-parallel (512 rows).
Each core computes a partial O^T [1024, 2048] for its batch; host sums the
two head-group partials per batch and transposes back to [S, D].

v2 pipeline (all matmul inputs bf16, cast on host):
  x^T via hardware DMA-XBAR transpose straight from DRAM (no PE transposes)
  Q^T/K^T = W^T x^T, V = x W_v -> persistent bf16 tiles
  attention per s-block, head pairs at PE row groups 0:64 / 64:128:
    paired score matmuls (K=64, concurrent row groups) -> one wide PSUM tile
    one merged exp over both heads' scores (ACT), causal-trimmed widths
    affine_select triangle mask only on the 128x128 diagonal squares
    AV accumulation with [V|1] ones column -> [65, 512] PSUM per head
    normalize: reciprocal_approx_fast + ones-matmul broadcast + DVE mul
  W_o projection + output DMA
  The W/V projections of s-block sb+1 and output projection of sb-1 are
  interleaved one matmul at a time into the attention loop of s-block sb to
  fill PE bubbles (keeps the HAM clock gate warm).
"""

from contextlib import ExitStack

import numpy as np
import ml_dtypes

import concourse.bass as bass
import concourse.mybir as mybir
import concourse.tile as tile
from concourse import bacc
from concourse.bass_utils import run_bass_kernel_spmd

F32 = mybir.dt.float32
BF16 = mybir.dt.bfloat16

S = 2048          # sequence length
D = 1024          # model dim
HL = 8            # local heads per core
DH = 64           # head dim
CL = HL * DH      # local cols (512)
SBS = 512         # s-block size
NSB = S // SBS    # 4 s-blocks
TS = 128          # tile size (partitions)
TTR = SBS // TS   # t-tiles per s-block (4)
N_CORES = 8

EXP = mybir.ActivationFunctionType.Exp
SCALE = 1.0 / 8.0  # 1/sqrt(DH)


def build_program():
    nc = bacc.Bacc(
        "TRN2", target_bir_lowering=False, debug=False, num_devices=N_CORES
    )
    x_d = nc.dram_tensor("x", [S, D], BF16, kind="ExternalInput").ap()
    wq_d = nc.dram_tensor("wq", [D, CL], BF16, kind="ExternalInput").ap()
    wk_d = nc.dram_tensor("wk", [D, CL], BF16, kind="ExternalInput").ap()
    wv_d = nc.dram_tensor("wv", [D, CL], BF16, kind="ExternalInput").ap()
    wo_d = nc.dram_tensor("wo", [CL, D], BF16, kind="ExternalInput").ap()
    out_d = nc.dram_tensor("out", [D, S], F32, kind="ExternalOutput").ap()

    with (
        tile.TileContext(nc) as tc,
        ExitStack() as ctx,
        nc.allow_low_precision(reason="bf16 attention, fp32 accumulation"),
    ):
        const = ctx.enter_context(tc.tile_pool(name="const", bufs=1))
        wpool = ctx.enter_context(tc.tile_pool(name="w", bufs=1))
        xtp = ctx.enter_context(tc.tile_pool(name="xt", bufs=1))
        ktp = ctx.enter_context(tc.tile_pool(name="kt", bufs=1))
        vp = ctx.enter_context(tc.tile_pool(name="v", bufs=1))
        qtp = ctx.enter_context(tc.tile_pool(name="qt", bufs=2))
        ptp = ctx.enter_context(tc.tile_pool(name="pt", bufs=3))
        ocp = ctx.enter_context(tc.tile_pool(name="oc", bufs=2))
        obp = ctx.enter_context(tc.tile_pool(name="ob", bufs=2))
        stp = ctx.enter_context(tc.tile_pool(name="st", bufs=4))
        mp = ctx.enter_context(tc.tile_pool(name="misc", bufs=8))
        psS = ctx.enter_context(tc.tile_pool(name="psS", bufs=2, space="PSUM"))
        psO = ctx.enter_context(tc.tile_pool(name="psO", bufs=2, space="PSUM"))
        psA = ctx.enter_context(tc.tile_pool(name="psA", bufs=2, space="PSUM"))

        # x^T: [128, 8 dtiles, 2048 s] via DMA-XBAR transpose, per (sb, dtile)
        # chunk so s-block 0's projections can start early.
        xt = xtp.tile([TS, 8, S], BF16)
        wq_s = wpool.tile([TS, 8, CL], BF16)
        wk_s = wpool.tile([TS, 8, CL], BF16)
        wv_s = wpool.tile([TS, 8, CL], BF16)
        # ALL dynamic DMAs serialize through one chain (cross-queue handoffs
        # cost ~1.3us each), so keep a single queue and order the chunks by
        # when the projection pipeline needs them: wq, x(sb0), wk, wv, then
        # the remaining x blocks, wo last.
        wo_s = wpool.tile([TS, 4, D], BF16)

        def x_chunks(sb):
            for j in range(8):
                nc.sync.dma_start(
                    out=xt[:, j, sb * SBS : (sb + 1) * SBS],
                    in_=x_d[sb * SBS : (sb + 1) * SBS, j * TS : (j + 1) * TS],
                    transpose=True,
                )

        for j in range(8):
            nc.sync.dma_start(wq_s[:, j, :], wq_d[j * TS : (j + 1) * TS, :])
        x_chunks(0)
        for w_s, w_d in ((wk_s, wk_d), (wv_s, wv_d)):
            for j in range(8):
                nc.sync.dma_start(
                    w_s[:, j, :], w_d[j * TS : (j + 1) * TS, :]
                )
        for sb in range(1, NSB):
            x_chunks(sb)
        for j in range(4):
            nc.sync.dma_start(wo_s[:, j, :], wo_d[j * TS : (j + 1) * TS, :])

        # K^T persistent: [128, 4 ctiles, 512]; head h -> ctile h//2, partition
        # offset (h%2)*64.  V persistent per head PAIR as [128, 4 stl, 4 pr,
        # 192]: cols 0:64 = v_even, col 64 = 1 (shared denominator column),
        # cols 65:128 = 1 (dead), cols 128:192 = v_odd.  The even head's AV
        # lhsT is cols 0:65 (M=65 -> AV rows 0..63, den row 64); the odd
        # head's is cols 64:192 (M=128 -> den row 0, AV rows 64..127), so the
        # pair's outputs concatenate to a full 128-partition tile with no
        # cross-partition moves.
        kt_r = [ktp.tile([TS, 4, SBS], BF16, name=f"kt_{r}") for r in range(NSB)]
        v_r = [
            vp.tile([TS, TTR, 4, 192], BF16, name=f"v_{r}") for r in range(NSB)
        ]
        for r in range(NSB):
            nc.vector.memset(v_r[r][:], 1.0)
        # all-ones [65+64, 64]: rows 0 and 64 serve as the [1,64] ones lhsT
        # for the reciprocal-broadcast matmuls at row groups 0 and 2.
        ones65 = const.tile([DH + 1, DH], F32)
        nc.vector.memset(ones65[:], 1.0)

        def proj_units(sb, qt):
            """B-stage for s-block sb: Q^T/K^T/V projections, one matmul per
            yield."""
            for ct in range(4):
                for w_s, isq in ((wq_s, True), (wk_s, False)):
                    ps = psA.tile(
                        [TS, SBS], F32, tag="ps", name=f"b_{sb}_{ct}_{int(isq)}"
                    )
                    for j in range(8):
                        nc.tensor.matmul(
                            ps[:],
                            w_s[:, j, ct * TS : (ct + 1) * TS],
                            xt[:, j, sb * SBS : (sb + 1) * SBS],
                            start=(j == 0),
                            stop=(j == 7),
                        )
                        yield
                    dst = qt if isq else kt_r[sb]
                    nc.vector.tensor_copy(dst[:, ct, :], ps[:])
            for stl in range(TTR):
                ps = psA.tile([TS, SBS], F32, tag="ps", name=f"bv_{sb}_{stl}")
                for j in range(8):
                    nc.tensor.matmul(
                        ps[:],
                        xt[:, j, sb * SBS + stl * TS : sb * SBS + (stl + 1) * TS],
                        wv_s[:, j, :],
                        start=(j == 0),
                        stop=(j == 7),
                    )
                    yield
                psv = ps[:].rearrange("p (pr two e) -> p pr two e", two=2, e=DH)
                nc.vector.tensor_copy(
                    v_r[sb][:, stl, :, 0:DH], psv[:, :, 0, :]
                )
                nc.vector.tensor_copy(
                    v_r[sb][:, stl, :, 2 * DH : 3 * DH], psv[:, :, 1, :]
                )

        def outproj_units(sb, oc, act_evac=False):
            """D-stage for s-block sb from its outcat^T tile, one matmul per
            yield.  act_evac=True moves the PSUM evacuation to the (by then
            idle) scalar engine and alternates output DMAs across both HWDGE
            queues — used for the final, serial output projection."""
            for mt in range(8):
                ps = psA.tile([TS, SBS], F32, tag="ps", name=f"d_{sb}_{mt}")
                for j in range(4):
                    nc.tensor.matmul(
                        ps[:],
                        wo_s[:, j, mt * TS : (mt + 1) * TS],
                        oc[:, j, :],
                        start=(j == 0),
                        stop=(j == 3),
                    )
                    yield
                ob = obp.tile([TS, SBS], F32)
                if act_evac:
                    nc.scalar.copy(ob[:], ps[:])
                else:
                    nc.vector.tensor_copy(ob[:], ps[:])
                nc.sync.dma_start(
                    out_d[mt * TS : (mt + 1) * TS, sb * SBS : (sb + 1) * SBS],
                    ob[:],
                )

        def chain(*gens):
            for g in gens:
                yield from g

        def take(gen, n):
            got = 0
            for _ in range(n):
                if next(gen, None) is None:
                    return got
                got += 1
            return got

        qt_tiles = [None] * NSB
        oc_tiles = [None] * NSB
        qt_tiles[0] = qtp.tile([TS, 4, SBS], BF16, tag="qt", name="qt_0")
        # s-block 0 projections run standalone (nothing to overlap yet)
        for _ in proj_units(0, qt_tiles[0]):
            pass

        for sb in range(NSB):
            qt = qt_tiles[sb]
            oc = ocp.tile([TS, 4, SBS], BF16, tag="oc", name=f"oc_{sb}")
            oc_tiles[sb] = oc
            # filler: output projection of sb-1, then projections of sb+1
            gens = []
            if sb >= 1:
                gens.append(outproj_units(sb - 1, oc_tiles[sb - 1]))
            if sb + 1 < NSB:
                qt_tiles[sb + 1] = qtp.tile(
                    [TS, 4, SBS], BF16, tag="qt", name=f"qt_{sb + 1}"
                )
                gens.append(proj_units(sb + 1, qt_tiles[sb + 1]))
            filler = chain(*gens)

            ntt = (sb + 1) * TTR
            pending = []
            for hp in range(4):
                ct = hp
                tts = list(range(sb * TTR, ntt)) + list(range(0, sb * TTR))
                pos = [
                    psO.tile([TS, SBS], F32, tag="po", name=f"po_{sb}_{hp}_{i}")
                    for i in range(2)
                ]

                def scores(ti):
                    tt = tts[ti]
                    k = tt - sb * TTR if ti < TTR else None
                    s0 = TS * k if k is not None else 0
                    w = psS.tile(
                        [TS, 2, SBS], F32, tag="sc", name=f"sc_{sb}_{hp}_{ti}"
                    )
                    for i in range(2):
                        poff = DH * i
                        nc.tensor.matmul(
                            w[:, i, s0:SBS],
                            kt_r[tt // TTR][
                                poff : poff + DH,
                                ct,
                                (tt % TTR) * TS : (tt % TTR + 1) * TS,
                            ],
                            qt[poff : poff + DH, ct, s0:SBS],
                            start=True,
                            stop=True,
                        )
                    pt = ptp.tile([TS, 2, SBS], BF16)
                    nc.scalar.activation(
                        pt[:, :, s0:SBS], w[:, :, s0:SBS], EXP, scale=SCALE
                    )
                    if k is not None:
                        for i in range(2):
                            # triangle mask on the diagonal 128x128 square:
                            # keep where col >= partition
                            nc.gpsimd.affine_select(
                                out=pt[:, i, s0 : s0 + TS],
                                in_=pt[:, i, s0 : s0 + TS],
                                compare_op=mybir.AluOpType.is_ge,
                                fill=0.0,
                                base=0,
                                channel_multiplier=-1,
                                pattern=[[1, TS]],
                            )
                    return pt, s0

                prev = scores(0)
                for ti in range(len(tts)):
                    nxt = scores(ti + 1) if ti + 1 < len(tts) else None
                    pt, s0 = prev
                    tt = tts[ti]
                    vpr = v_r[tt // TTR][:, tt % TTR, hp]
                    nc.tensor.matmul(
                        pos[0][0 : DH + 1, s0:SBS],
                        vpr[:, 0 : DH + 1],
                        pt[:, 0, s0:SBS],
                        start=(ti == 0), stop=(ti == len(tts) - 1),
                    )
                    nc.tensor.matmul(
                        pos[1][:, s0:SBS],
                        vpr[:, DH:],
                        pt[:, 1, s0:SBS],
                        start=(ti == 0), stop=(ti == len(tts) - 1),
                    )
                    if pending:
                        pending.pop(0)()
                    take(filler, 2)
                    prev = nxt

                # Reciprocal of the denominator row straight from PSUM first
                # (fast approx, ~0.7us) so the deferred bc matmul on PE is
                # never blocked; then evacuate the [64, 512] accumulators so
                # the pos PSUM banks free.  Normalize tail (broadcast/multiply)
                # is deferred into the next head pair's loop.
                # Denominator rows (even head: pos0 row 64; odd head: pos1
                # row 0) -> one [2,512] reciprocal, issued first so the
                # deferred bc matmul on PE is never blocked; then evacuate
                # the pair's AV halves into one [128,512] tile.  The last
                # head pair evacuates on the by-then-idle scalar engine.
                last = sb == NSB - 1 and hp == 3
                # den rows live at partitions {0, 64} (engine writes must
                # start at a 0/32/64 partition base); the recip runs over all
                # 128 partitions (same cost, rows 1..63 are dead).
                den = mp.tile([TS, SBS], F32, tag="den", name=f"den_{sb}_{hp}")
                nc.vector.tensor_copy(den[0:1, :], pos[0][DH : DH + 1, :])
                nc.vector.tensor_copy(den[DH : DH + 1, :], pos[1][0:1, :])
                rs = mp.tile([TS, SBS], F32, tag="rs", name=f"rs_{sb}_{hp}")
                nc.vector.reciprocal_approx_fast(out=rs[:], in_=den[:])
                pn = stp.tile([TS, SBS], F32, tag="pn")
                if last:
                    nc.scalar.copy(pn[0:DH, :], pos[0][0:DH, :])
                    nc.scalar.copy(pn[DH:TS, :], pos[1][DH:TS, :])
                else:
                    nc.vector.tensor_copy(pn[0:DH, :], pos[0][0:DH, :])
                    nc.vector.tensor_copy(pn[DH:TS, :], pos[1][DH:TS, :])

                def norm_tail(pn, rs, ct=ct, sb=sb, hp=hp, oc=oc):
                    bc = psA.tile(
                        [TS, SBS], F32, tag="ps", name=f"bc_{sb}_{hp}"
                    )
                    # two concurrent-capable broadcasts: (row grp 0 -> cols
                    # 0:64) and (row grp 2 -> cols 64:128)
                    nc.tensor.matmul(
                        bc[0:DH, :], ones65[0:1, :], rs[0:1, :],
                        start=True, stop=True,
                    )
                    nc.tensor.matmul(
                        bc[DH:TS, :], ones65[DH : DH + 1, :],
                        rs[DH : DH + 1, :],
                        start=True, stop=True,
                    )
                    nc.vector.tensor_mul(oc[:, ct, :], pn[:], bc[:])

                pending.append(lambda pn=pn, rs=rs: norm_tail(pn, rs))

            # flush deferred normalize tails for the last head pair
            for u in pending:
                u()
            pending = []
            # drain remaining fillers before the next s-block needs qt/kt/v
            for _ in filler:
                pass

        # final output projection (nothing left to overlap with)
        for _ in outproj_units(NSB - 1, oc_tiles[NSB - 1], act_evac=True):
            pass

    nc.compile()
    return nc


_prog_cache = {}


def _get_program():
    if "p" not in _prog_cache:
        _prog_cache["p"] = build_program()
    return _prog_cache["p"]


def make_in_maps(inputs):
    bf = ml_dtypes.bfloat16
    x = np.asarray(inputs["x"], np.float32)
    wq = np.asarray(inputs["W_q"], np.float32)
    wk = np.asarray(inputs["W_k"], np.float32)
    wv = np.asarray(inputs["W_v"], np.float32)
    wo = np.asarray(inputs["W_o"], np.float32)
    in_maps = []
    for c in range(N_CORES):
        b, g = c // 2, c % 2
        cs = slice(g * CL, (g + 1) * CL)
        in_maps.append(
            {
                "x": np.ascontiguousarray(x[b]).astype(bf),
                "wq": np.ascontiguousarray(wq[:, cs]).astype(bf),
                "wk": np.ascontiguousarray(wk[:, cs]).astype(bf),
                "wv": np.ascontiguousarray(wv[:, cs]).astype(bf),
                "wo": np.ascontiguousarray(wo[cs, :]).astype(bf),
            }
        )
    return in_maps


def run(inputs, trace=False, **kwargs):
    nc = _get_program()
    res = run_bass_kernel_spmd(
        nc, make_in_maps(inputs), core_ids=list(range(N_CORES)),
        trace=trace, **kwargs
    )
    outs = [res.results[c]["out"] for c in range(N_CORES)]
    full = np.stack(
        [(outs[2 * b] + outs[2 * b + 1]).T for b in range(4)]
    ).astype(np.float32)
    return full, res


def kernel(**inputs) -> np.ndarray:
    out, _ = run(inputs)
    return out

